# revision 5
# baseline (speedup 1.0000x reference)
"""Causal self-attention (B=4, T=2048, C=768, H=12) on 8 TRN2 NeuronCores.

Sharding: data-parallel over batch (4) x tensor-parallel over heads (2 groups
of 6).  Core c handles batch c//2, head-group c%2.  Each core computes its
QKV projection slice, causal flash-attention for its 6 heads, and a partial
output projection; the host sums the two head-group partials per batch and
adds b_proj.

v2/v3 changes vs the previous baseline (cost-model sim 188 -> 167us/body;
measured ~390 -> ~250-340us/body via the n_reps differential):
  - fp8 q/k for each fc-pair (q-chunk m, k-chunk m+3) co-located in one
    [128, 3584] tile, laid out as interleaved 512-token blocks
    [q(512:1024)|k(512:1024)|q(1024:1536)|k(1024:1536)|q(1536:2048)|
     k(1536:2048)|k(0:512)] so each DoubleRow partition fold covers q+k of
    BOTH heads of the pair over a contiguous col range: 24 fold DMAs -> 20,
    with the critical first fold (A: t in [512,1024)) also covering head 1.
  - output is bf16 and batched 4 token-tiles per DMA (16 f32 DMAs -> 4
    bf16 DMAs); host combines partials in f32.
  - software-pipelined schedule: after the head-0 warmup, every PV slot's
    post-filler emits exactly one qk_block of a later phase (live pt tiles
    pinned at the pool's 16), so ScalarE's exp stream -- the busiest engine
    at ~115us -- never drains at head boundaries.  All icp0 phases run
    before icp1 so the output projection rides the icp1 passes.

fp8 acceleration (e4m3, weights pre-scaled x64 host-side so w~0.02 clears the
fp8 subnormal floor; the bias-add divides by 64):
  - q,k projection for tokens t>=512 runs as fp8 DoubleRow matmuls pairing
    contraction chunks (4x PE rate); t<512 stays bf16 and is stored twice
    (bf16 + fp8) because early tokens' softmax has no averaging to damp
    quantization noise.
  - S^T = K^T-tile @ Q for query rows i>=512 runs as fp8 DoubleRow with the
    64-wide head dim split into 2 planes of 32 on the partition axis (2x PE
    rate, no wasted plane); rows i<512 stay bf16 from the exactly-projected
    q,k.
  - v projection for tokens t>=512 runs as fp8 DoubleRow off the
    already-resident fp8 x (the bf16 x stream for t>=512 is dropped
    entirely -- v was its only consumer): v1 holds WS*(v+b_v), the
    ones-block denominator stays unscaled, and the output-projection
    copy divides by WS.  Keys j<512 keep the bf16-exact v so early query
    rows (no averaging) are untouched; rows i>=512 average >=512 keys,
    damping the ~2.4% fp8 v noise to ~1e-3 of the output.
  - PV and both output projections stay bf16: p carries relative element
    noise straight to the output (no n_eff damping), so fp8 there would
    blow the error budget (and folding p across partitions for DoubleRow
    is not expressible without per-tile DMAs).
Everything else matches the bf16 baseline: v carries a 64-wide ones block so
the PV matmul emits softmax denominators replicated across 64 partitions
(normalize = reciprocal + multiply on VectorE); exp on ScalarE straight out
of PSUM; causal triangle via affine_select on GpSimd; output projection
rides inside the last head's jt loop.
"""
import sys

try:
    import concourse  # noqa: F401
except ImportError:
    sys.path.insert(0, "/opt/trn_rl_repo")

import numpy as np
import concourse.bacc as bacc
import concourse.mybir as mybir
import concourse.tile as tile
from concourse.bass_utils import run_bass_kernel_spmd

f32 = mybir.dt.float32
bf16 = mybir.dt.bfloat16
f8 = mybir.dt.float8e4
IN_DT = bf16
Exp = mybir.ActivationFunctionType.Exp
DR = mybir.MatmulPerfMode.DoubleRow
MULT = mybir.AluOpType.mult
ADD = mybir.AluOpType.add

B, T, C, H = 4, 2048, 768, 12
FC_ORDER = [0, 3, 1, 4, 2, 5]   # host lays w_qk/b_qk columns out in this
FC_POS = {fc: i for i, fc in enumerate(FC_ORDER)}  # feature-chunk order
HD = 64          # head dim
GW = 384         # head-group width (6 heads)
SCALE = HD ** -0.5
WS = 64.0        # host-side fp8 weight pre-scale


def _qcol(c):
    # q fp8 column for q-coord c = i - 512, c in [0, 1536)
    return 1024 * (c // 512) + c % 512


def _kcol(j):
    # k fp8 column for token j in [0, 2048)
    if j < 512:
        return 3072 + j
    return 512 + 1024 * ((j - 512) // 512) + (j - 512) % 512


def _emit(tc, xt, xt8, w_qk8, w_qkb, w_v, w_v8, b_qk, b_v, w_p, out, n_reps=1):
    nc = tc.nc

    with tc.tile_pool(name="const", bufs=1) as const, \
         tc.tile_pool(name="qkv", bufs=1) as qkv, \
         tc.tile_pool(name="psp", bufs=2, space="PSUM") as psp, \
         tc.tile_pool(name="pog", bufs=4, space="PSUM") as pog, \
         tc.tile_pool(name="ptp", bufs=16) as ptp, \
         tc.tile_pool(name="nrm", bufs=4) as nrm, \
         tc.tile_pool(name="ob", bufs=2) as ob:
        bqk_all = const.tile([128, 6], f32, name="bqk")
        bqk_sb = [bqk_all[:, fc:fc + 1] for fc in range(6)]
        bv_sb = const.tile([128, GW], f32, name="bv")
        ones6 = const.tile([128, 6], f32, name="ones6")
        nc.vector.memset(ones6, 1.0)
        wp_all = const.tile([128, 3, C], IN_DT, name="wp")
        wp_sb = [wp_all[:, fc, :] for fc in range(3)]

        def load_consts():
            nc.sync.dma_start(
                out=bqk_all, in_=b_qk[:].rearrange("(fc p) -> p fc", p=128))
            nc.sync.dma_start(
                out=bv_sb,
                in_=b_v[:][None, :].partition_broadcast(128).opt(keep_dims={0}))

        def load_wp():
            nc.sync.dma_start(
                out=wp_all, in_=w_p[:, :].rearrange("(fc p) n -> p fc n", p=128))

        # ---- persistent per-rep tensors
        # bf16 q,k for t<512 (exact early path)
        qkTb = [qkv.tile([128, 512], IN_DT, name=f"qkTb{fc}") for fc in range(6)]
        # fp8 q (t>=512) + k (all t) per fc-pair, fold-friendly layout
        qk8c = [qkv.tile([128, 3584], f8, name=f"qk8c{m}") for m in range(3)]
        # head-dim-folded fp8 copies: [p, head-in-pair, plane, col] where
        # source partition is 64*hh + 32*pl + p
        kqf8 = [qkv.tile([32, 2, 2, 3584], f8, name=f"kqf8_{m}") for m in range(3)]
        v1 = [qkv.tile([128, 6, 128], IN_DT, name=f"v1_{tt}") for tt in range(16)]
        yT = [qkv.tile([128, T], IN_DT, name=f"yT{fc}") for fc in range(3)]

        for tt in range(16):
            nc.gpsimd.memset(v1[tt][:, :, 64:128], 1.0)

        for _ in range(n_reps):
            with tc.tile_pool(name="xw", bufs=1) as xw:
                w8_all = xw.tile([128, 6, 2 * GW], f8, name="w8")
                xt8_all = xw.tile([128, 6, T], f8, name="xt8")
                nc.sync.dma_start(out=w8_all, in_=w_qk8[:, :, :])
                # t4=1 block first: the first attention tiles need only q,k
                # for t in [512,1024), so the first fold fires after 1/3 of
                # the fp8 x stream.  The biases ride right behind (the first
                # bias-add gates the first fold); the rest of the fp8 x
                # stream moves into load_bulk_a so its transfer never sits
                # ahead of the critical first fold on the queue.
                nc.sync.dma_start(out=xt8_all[:, :, 512:1024],
                                  in_=xt8[:, :, 512:1024])
                nc.sync.dma_start(out=xt8_all[:, :, 1024:2048],
                                  in_=xt8[:, :, 1024:2048])
                load_consts()
                wv8_all = xw.tile([128, 6, GW], f8, name="wv8")
                nc.sync.dma_start(out=wv8_all, in_=w_v8[:, :, :])
                wv_all = xw.tile([128, 6, GW], IN_DT, name="wv")
                nc.sync.dma_start(
                    out=wv_all, in_=w_v[:, :].rearrange("(cc p) f -> p cc f", p=128))
                xt_all = xw.tile([128, 6, 512], IN_DT, name="xtb")
                xt_sb = [xt_all[:, cc, :] for cc in range(6)]
                wb_all = xw.tile([128, 6, 2 * GW], IN_DT, name="wb")

                # The bulk bf16 stream loads in three groups so its
                # serialized DMA transfers never sit ahead of the
                # attention-critical fp8 folds: group a rides the sync-queue
                # FIFO right behind the first fold (whose sem wait blocks
                # the queue), groups b/c are gated by marker copies that
                # READ a fold output and WRITE a cell of the load target.
                # wb splits at col 256: FC_ORDER puts fc 0,3 in [0:256], so
                # wb_a alone unblocks the early qk8b.
                def load_bulk_a():
                    nc.sync.dma_start(out=xt_all[:, :, 0:512],
                                      in_=xt[:, :, 0:512])
                    nc.sync.dma_start(out=wb_all[:, :, 0:256],
                                      in_=w_qkb[:, :, 0:256])

                def load_bulk_c():
                    nc.vector.tensor_copy(wb_all[0:1, 0, 256:257],
                                          kqf8[0][0:1, 1, 1, 3072:3073])
                    nc.sync.dma_start(out=wb_all[:, :, 256:2 * GW],
                                      in_=w_qkb[:, :, 256:2 * GW])
                    load_wp()

                def qk8f1(fc, t4):
                    # fp8 DoubleRow projection for one 512-token block
                    pos = FC_POS[fc]
                    m = fc if fc < 3 else fc - 3
                    base = 1024 * (t4 - 1) + (0 if fc < 3 else 512)
                    pq = pog.tile([128, 512], f32, name="po")
                    for ccp in range(3):
                        nc.tensor.matmul(
                            pq,
                            w8_all[:, 2 * ccp:2 * ccp + 2,
                                   128 * pos:128 * (pos + 1)],
                            xt8_all[:, 2 * ccp:2 * ccp + 2,
                                    512 * t4:512 * (t4 + 1)],
                            start=(ccp == 0), stop=(ccp == 2),
                            perf_mode=DR)
                    nc.vector.tensor_scalar(
                        qk8c[m][:, base:base + 512],
                        pq, 1.0 / WS, bqk_sb[pos], MULT, ADD)

                def qk8f(fc):
                    for t4 in range(1, 4):
                        qk8f1(fc, t4)

                def qk8b(fc):
                    # bf16 projection for t in [0, 512)
                    pos = FC_POS[fc]
                    pq = pog.tile([128, 512], f32, name="po")
                    for cc in range(6):
                        nc.tensor.matmul(
                            pq, wb_all[:, cc, 128 * pos:128 * (pos + 1)],
                            xt_sb[cc][:, 0:512],
                            start=(cc == 0), stop=(cc == 5))
                    nc.vector.tensor_scalar_add(qkTb[fc], pq, bqk_sb[pos])
                    if fc >= 3:  # k also needed in fp8 for rows i>=512
                        nc.vector.tensor_scalar_add(
                            qk8c[fc - 3][:, 3072:3584], pq, bqk_sb[pos])

                def fold(m, lo, hi):
                    # SBUF->SBUF partition fold into DoubleRow form for both
                    # heads of pair m, col range [lo, hi): partition
                    # 64*hh + 32*pl + p  ->  [p, hh, pl, col].  One DMA per
                    # (head, plane): SBUF APs cannot split the partition dim.
                    for hh in range(2):
                        for pl in range(2):
                            r = 64 * hh + 32 * pl
                            nc.sync.dma_start(
                                out=kqf8[m][:, hh, pl, lo:hi],
                                in_=qk8c[m][r:r + 32, lo:hi])

                def foldA(m):
                    fold(m, 0, 1024)      # q+k for t in [512, 1024)

                def foldB(m):
                    fold(m, 1024, 3072)   # q+k for t in [1024, 2048)

                def foldC(m):
                    fold(m, 3072, 3584)   # k for t in [0, 512)

                def foldAll(m):
                    fold(m, 0, 3584)

                def v_chunk(tt):
                    # v1 holds WS*(v + b_v) (b_v arrives WS-prescaled); the
                    # output-projection copy divides by WS.  tokens >= 512
                    # run as fp8 DoubleRow off the already-resident xt8
                    # (late query rows average >=512 keys, damping the fp8
                    # v noise); tokens < 512 stay bf16-exact for the early
                    # rows, scaled to match.
                    pv = pog.tile([128, GW], f32, name="po")
                    if tt >= 4:
                        for ccp in range(3):
                            nc.tensor.matmul(
                                pv,
                                xt8_all[:, 2 * ccp:2 * ccp + 2,
                                        128 * tt:128 * (tt + 1)],
                                wv8_all[:, 2 * ccp:2 * ccp + 2, :],
                                start=(ccp == 0), stop=(ccp == 2),
                                perf_mode=DR)
                        v3 = v1[tt]
                        nc.vector.tensor_add(
                            v3[:, :, 0:64],
                            pv.rearrange("p (h e) -> p h e", e=64),
                            bv_sb.rearrange("p (h e) -> p h e", e=64))
                    else:
                        for cc in range(6):
                            nc.tensor.matmul(
                                pv, xt_sb[cc][:, 128 * tt:128 * (tt + 1)],
                                wv_all[:, cc, :],
                                start=(cc == 0), stop=(cc == 5))
                        v3 = v1[tt]
                        nc.vector.tensor_scalar_mul(
                            v3[:, :, 0:64],
                            pv.rearrange("p (h e) -> p h e", e=64), WS)
                        nc.vector.tensor_add(
                            v3[:, :, 0:64], v3[:, :, 0:64],
                            bv_sb.rearrange("p (h e) -> p h e", e=64))

                o_group = {}

                def proj_range(tt_lo, tt_hi):
                    for tt in range(tt_lo, tt_hi):
                        g = tt // 4
                        if g not in o_group:
                            o_group[g] = ob.tile([128, 4, C], IN_DT, name="o")
                        o_sb = o_group[g]
                        for nh in range(2):
                            pp = pog.tile([128, GW], f32, name="po")
                            for fc in range(3):
                                nc.tensor.matmul(
                                    pp, yT[fc][:, 128 * tt:128 * (tt + 1)],
                                    wp_sb[fc][:, GW * nh:GW * (nh + 1)],
                                    start=(fc == 0), stop=(fc == 2))
                            nc.vector.tensor_scalar_mul(
                                o_sb[:, tt % 4, GW * nh:GW * (nh + 1)], pp,
                                1.0 / WS)
                        if tt % 4 == 3:
                            nc.sync.dma_start(
                                out=out[512 * g:512 * (g + 1), :].rearrange(
                                    "(q p) c -> p q c", p=128),
                                in_=o_group.pop(g))

                pts = {}    # (h, icp, jt) -> pt tile, from a QK pass

                def qk_block(h, icp, jt):
                    # S^T tile + exp + causal mask for one (head, i-range, jt)
                    r0 = 64 * (h % 2)
                    qb_t, kb_t = qkTb[h // 2], qkTb[3 + h // 2]
                    qf = kqf8[h // 2][:, h % 2]     # [32, 2, 3584]
                    i_lo = 1024 * icp
                    j0 = 128 * jt
                    kc = _kcol(j0)
                    vs = max(j0 - i_lo, 0)
                    ps_t = psp.tile([128, 1024], f32, name="ps")
                    if icp == 0:
                        if vs < 512:
                            nc.tensor.matmul(
                                ps_t[:, vs:512],
                                kb_t[r0:r0 + 64, j0:j0 + 128],
                                qb_t[r0:r0 + 64, vs:512],
                                start=True, stop=True)
                            nc.tensor.matmul(
                                ps_t[:, 512:1024], qf[:, :, kc:kc + 128],
                                qf[:, :, _qcol(0):_qcol(0) + 512],
                                start=True, stop=True, perf_mode=DR)
                        else:
                            a = vs - 512
                            nc.tensor.matmul(
                                ps_t[:, vs:1024], qf[:, :, kc:kc + 128],
                                qf[:, :, _qcol(a):_qcol(a) + (512 - a)],
                                start=True, stop=True, perf_mode=DR)
                    else:
                        # i in [1024, 2048) -> q-coord (i - 512)
                        if vs < 512:
                            a = 512 + vs
                            nc.tensor.matmul(
                                ps_t[:, vs:512],
                                qf[:, :, kc:kc + 128],
                                qf[:, :, _qcol(a):_qcol(a) + (512 - vs)],
                                start=True, stop=True, perf_mode=DR)
                            nc.tensor.matmul(
                                ps_t[:, 512:1024], qf[:, :, kc:kc + 128],
                                qf[:, :, _qcol(1024):_qcol(1024) + 512],
                                start=True, stop=True, perf_mode=DR)
                        else:
                            a = 512 + vs
                            nc.tensor.matmul(
                                ps_t[:, vs:1024], qf[:, :, kc:kc + 128],
                                qf[:, :, _qcol(a):_qcol(a) + (1024 - vs)],
                                start=True, stop=True, perf_mode=DR)
                    pt_t = ptp.tile([128, 1024], IN_DT, name="pt")
                    nc.scalar.activation(
                        pt_t[:, vs:1024], ps_t[:, vs:1024], Exp, scale=SCALE)
                    if j0 >= i_lo:
                        # triangular mask on the diagonal block:
                        # keep where (i - j) = f - p >= 0, else 0
                        nc.gpsimd.affine_select(
                            out=pt_t[:, vs:vs + 128], in_=pt_t[:, vs:vs + 128],
                            compare_op=mybir.AluOpType.is_ge, fill=0.0,
                            base=0, pattern=[[1, 128]], channel_multiplier=-1)
                    pts[(h, icp, jt)] = pt_t

                def qk_pass(h, icp, jt_list, fillers=()):
                    for slot, jt in enumerate(jt_list):
                        qk_block(h, icp, jt)
                        if slot < len(fillers) and fillers[slot] is not None:
                            fillers[slot]()

                def _run(f):
                    if f is None:
                        return
                    if callable(f):
                        f()
                    else:
                        for g in f:
                            _run(g)

                def pv_pass(h, icp, jt_order, pv_fillers=(), post_fillers=()):
                    # PV accumulation + normalization over tiles a QK pass
                    # produced.  first/last contributing jt per accumulator
                    # half, in emission order (PV start/stop + norm points).
                    # pv_fillers run before each slot's pt consumption,
                    # post_fillers after it (posts emitting the next phase's
                    # qk_blocks go there so the pt pool never exceeds its
                    # 16-buffer depth).
                    r0 = 64 * (h % 2)
                    i_lo = 1024 * icp
                    ends = []
                    for half in range(2):
                        hi = 512 * (half + 1)
                        contrib = [jt for jt in jt_order
                                   if max(128 * jt - i_lo, 0) < hi]
                        ends.append((contrib[0], contrib[-1]))
                    po2 = [pog.tile([128, 512], f32, name="po") for _ in range(2)]
                    for slot, jt in enumerate(jt_order):
                        vs = max(128 * jt - i_lo, 0)
                        if slot < len(pv_fillers):
                            _run(pv_fillers[slot])
                        pt_t = pts.pop((h, icp, jt))
                        for half in range(2):
                            hi = 512 * (half + 1)
                            first_jt, stop_jt = ends[half]
                            if vs < hi:
                                rl = max(vs, 512 * half)
                                nc.tensor.matmul(
                                    po2[half][:, rl - 512 * half:512],
                                    v1[jt][:, h, :], pt_t[:, rl:hi],
                                    start=(jt == first_jt), stop=(jt == stop_jt))
                            if jt == stop_jt:
                                # normalize this half as soon as its
                                # accumulation closes: po rows 64:128 hold
                                # the denominator replicated across 64
                                # partitions (ones block) -> recip + mul.
                                bc_sb = nrm.tile([64, 512], f32, name="bc")
                                nc.vector.reciprocal(bc_sb, po2[half][64:128, :])
                                nc.vector.tensor_mul(
                                    yT[h // 2][r0:r0 + 64,
                                               i_lo + 512 * half:
                                               i_lo + 512 * (half + 1)],
                                    po2[half][0:64, :], bc_sb)
                        if slot < len(post_fillers):
                            _run(post_fillers[slot])

                def F(fn, *a):
                    return lambda: fn(*a)

                # icp-major schedule: all i<1024 phases first, then i>=1024.
                # Non-attention PE work rides inside the jt loops (fillers)
                # so ScalarE's exp stream never waits on a bulk PE phase.
                # att(0,0) runs its fp8-only diagonal tiles (jt 4..7) first:
                # they need only the fp8 projection + the A fold, so exp
                # starts while the bf16 early path is still streaming in.
                def head0_late():
                    # bf16 early projection + C fold: waits on the bf16
                    # x/w stream, so it rides inside att(0,0) after the
                    # fp8-only diagonal tiles instead of gating them
                    qk8b(3)
                    qk8b(0)
                    foldC(0)
                    load_bulk_c()

                # minimal chain to the first exp: one t4 block of the fp8
                # projection for fc 3 and 0, then the A fold of pair 0
                # (heads 0+1, q+k for t in [512,1024)).  The full fc0/fc3
                # fp8 projection + B fold follow so head 0's big icp1 exp
                # batch (12 tiles, no bf16 deps) keeps ScalarE busy while
                # the bf16 stream loads.
                qk8f1(3, 1)
                qk8f1(0, 1)
                foldA(0)
                load_bulk_a()
                qk8f1(3, 2)
                qk8f1(0, 2)
                qk8f1(3, 3)
                qk8f1(0, 3)
                foldB(0)

                def QKB(h, icp, jts):
                    return [F(qk_block, h, icp, jt) for jt in jts]

                DIAG = [4, 5, 6, 7, 0, 1, 2, 3]        # head-0 icp0 pv order
                FULL = list(range(4, 16)) + [0, 1, 2, 3]  # head-0 icp1 order
                NAT8 = list(range(8))     # heads 1-5: all tiles ready, so
                NAT16 = list(range(16))   # natural order closes norms early

                # Software-pipelined schedule: after the head-0 warmup, every
                # PV slot's post-filler emits exactly one qk_block of a later
                # phase, so ScalarE's exp stream never drains at head
                # boundaries and the live pt count stays at the pool's 16.
                # icp0 phases all run before the icp1 phases so the output
                # projection (which needs every head's yT rows) can ride the
                # icp1 passes.
                qk_pass(0, 0, [4, 5, 6, 7])
                qk_pass(0, 1, list(range(4, 16)),
                        fillers=[None] * 6 + [head0_late])
                pv_pass(0, 0, DIAG,
                        pv_fillers=[[F(v_chunk, 4), F(qk8f, 1)],
                                    [F(v_chunk, 5), F(qk8f, 4)],
                                    F(v_chunk, 6), F(v_chunk, 7),
                                    F(v_chunk, 0), F(v_chunk, 1),
                                    F(v_chunk, 2), F(v_chunk, 3)],
                        post_fillers=QKB(0, 0, [0, 1, 2, 3])
                        + QKB(0, 1, [0, 1, 2, 3]))
                pv_pass(0, 1, FULL,
                        pv_fillers=[[F(v_chunk, 8), F(qk8b, 1)],
                                    [F(v_chunk, 9), F(qk8b, 4)],
                                    [F(v_chunk, 10), F(foldAll, 1)],
                                    F(v_chunk, 11), F(v_chunk, 12),
                                    F(v_chunk, 13), F(v_chunk, 14),
                                    F(v_chunk, 15)],
                        post_fillers=QKB(1, 0, NAT8) + QKB(2, 0, NAT8))
                pv_pass(1, 0, NAT8,
                        pv_fillers=[F(qk8f, 2), F(qk8f, 5), F(qk8b, 2),
                                    F(qk8b, 5), F(foldAll, 2)],
                        post_fillers=QKB(3, 0, NAT8))
                pv_pass(2, 0, NAT8, post_fillers=QKB(4, 0, NAT8))
                pv_pass(3, 0, NAT8, post_fillers=QKB(5, 0, NAT8))
                pv_pass(4, 0, NAT8, post_fillers=QKB(1, 1, NAT16[:8]))
                pv_pass(5, 0, NAT8, post_fillers=QKB(1, 1, NAT16[8:]))
                pv_pass(1, 1, NAT16,
                        pv_fillers=[F(proj_range, 0, 1), None, None, None,
                                    F(proj_range, 1, 2)],
                        post_fillers=QKB(2, 1, NAT16))
                pv_pass(2, 1, NAT16,
                        pv_fillers=[F(proj_range, 2, 3), None, None, None,
                                    F(proj_range, 3, 4)],
                        post_fillers=QKB(3, 1, NAT16))
                pv_pass(3, 1, NAT16,
                        pv_fillers=[F(proj_range, 4, 5), None, None, None,
                                    F(proj_range, 5, 6)],
                        post_fillers=QKB(4, 1, NAT16))
                pv_pass(4, 1, NAT16,
                        pv_fillers=[F(proj_range, 6, 7), None, None, None,
                                    F(proj_range, 7, 8)],
                        post_fillers=QKB(5, 1, NAT16))
                # proj 8..11 need head 5's i[1024:1536) half, which closes
                # at its jt 11 (slot 11 in natural order) -- ride slots 12+.
                pv_pass(5, 1, NAT16,
                        pv_fillers=[None] * 12 + [F(proj_range, 8, 9),
                                                  F(proj_range, 9, 10),
                                                  F(proj_range, 10, 11),
                                                  F(proj_range, 11, 12)])
                proj_range(12, 16)


_CACHE = {}


def _build(n_reps=1):
    key = ("nc", n_reps)
    if key in _CACHE:
        return _CACHE[key]
    nc = bacc.Bacc("TRN2", target_bir_lowering=False, debug=False)
    xt = nc.dram_tensor("xt", [128, 6, T], IN_DT, kind="ExternalInput")
    xt8 = nc.dram_tensor("xt8", [128, 6, T], f8, kind="ExternalInput")
    w_qk8 = nc.dram_tensor("w_qk8", [128, 6, 2 * GW], f8, kind="ExternalInput")
    w_qkb = nc.dram_tensor("w_qkb", [128, 6, 2 * GW], IN_DT, kind="ExternalInput")
    w_v = nc.dram_tensor("w_v", [C, GW], IN_DT, kind="ExternalInput")
    w_v8 = nc.dram_tensor("w_v8", [128, 6, GW], f8, kind="ExternalInput")
    b_qk = nc.dram_tensor("b_qk", [2 * GW], f32, kind="ExternalInput")
    b_v = nc.dram_tensor("b_v", [GW], f32, kind="ExternalInput")
    w_p = nc.dram_tensor("w_p", [GW, C], IN_DT, kind="ExternalInput")
    out = nc.dram_tensor("out", [T, C], IN_DT, kind="ExternalOutput")
    with tile.TileContext(nc) as tc:
        _emit(tc, xt[:, :, :], xt8[:, :, :], w_qk8[:, :, :], w_qkb[:, :, :],
              w_v[:, :], w_v8[:, :, :], b_qk[:], b_v[:], w_p[:, :], out[:, :],
              n_reps=n_reps)
    nc.compile()
    _CACHE[key] = nc
    return nc


def make_in_maps(x, w_attn, b_attn, w_proj):
    import ml_dtypes
    nbf16 = ml_dtypes.bfloat16
    nf8 = ml_dtypes.float8_e4m3
    x = np.asarray(x, dtype=np.float32)
    w_attn = np.asarray(w_attn, dtype=np.float32)
    b_attn = np.asarray(b_attn, dtype=np.float32)
    w_proj = np.asarray(w_proj, dtype=np.float32)
    # shared per-batch / per-head-group tensors computed once, not per core
    xts = [np.ascontiguousarray(
               x[b].T.reshape(6, 128, T).transpose(1, 0, 2)).astype(nbf16)
           for b in range(B)]
    xt8s = [np.ascontiguousarray(
                x[b].T.reshape(6, 128, T).transpose(1, 0, 2)).astype(nf8)
            for b in range(B)]
    per_s = []
    for s in range(2):
        q = slice(GW * s, GW * (s + 1))
        k = slice(C + GW * s, C + GW * (s + 1))
        v = slice(2 * C + GW * s, 2 * C + GW * (s + 1))
        wqk_full = np.concatenate([w_attn[:, q], w_attn[:, k]], axis=1)
        bqk_full = np.concatenate([b_attn[q], b_attn[k]])
        wqk_ord = np.concatenate(
            [wqk_full[:, 128 * fc:128 * (fc + 1)] for fc in FC_ORDER], axis=1)
        bqk_ord = np.concatenate(
            [bqk_full[128 * fc:128 * (fc + 1)] for fc in FC_ORDER])
        # [768, 768] -> [128, 6, 768] (contraction chunk planes)
        wqk_p = wqk_ord.reshape(6, 128, 2 * GW).transpose(1, 0, 2)
        per_s.append({
            "w_qk8": np.ascontiguousarray((WS * wqk_p).astype(nf8)),
            "w_qkb": np.ascontiguousarray(wqk_p.astype(nbf16)),
            "w_v": np.ascontiguousarray(w_attn[:, v].astype(nbf16)),
            "w_v8": np.ascontiguousarray(
                (WS * w_attn[:, v]).reshape(6, 128, GW).transpose(1, 0, 2)
                .astype(nf8)),
            "b_qk": np.ascontiguousarray(bqk_ord),
            "b_v": np.ascontiguousarray(WS * b_attn[v]),
            "w_p": np.ascontiguousarray(
                w_proj[GW * s:GW * (s + 1), :].astype(nbf16)),
        })
    return [{"xt": xts[c // 2], "xt8": xt8s[c // 2], **per_s[c % 2]}
            for c in range(8)]


def combine_outputs(results, b_proj):
    b_proj = np.asarray(b_proj, dtype=np.float32)
    outs = [results[c]["out"].astype(np.float32) for c in range(8)]
    y = np.stack([outs[2 * b] + outs[2 * b + 1] for b in range(B)])
    return (y + b_proj[None, None, :]).astype(np.float32)


def kernel(x, w_attn, b_attn, w_proj, b_proj, last_k_no_attend=0, window_size=0):
    # last_k_no_attend / window_size are 0 in this problem (no-op branch).
    nc = _build()
    in_maps = make_in_maps(x, w_attn, b_attn, w_proj)
    res = run_bass_kernel_spmd(nc, in_maps, list(range(8)))
    return combine_outputs(res.results, b_proj)
